# revision 22
# baseline (speedup 1.0000x reference)
"""CBAM-loss (LDAM-style margin cross-entropy) Trainium2 kernel.

Contract: kernel(**inputs) takes the FULL unsharded inputs
(x [32768, 1000] f32, targets [32768] int, cls_num_list [1000] f32,
class_difficulty [1000] f32, epoch int) and returns the scalar mean
loss (float32), matching:

    m_list1 = margins(cls_num_list, class_difficulty, epoch)   # [C]
    out = x; out[i, t_i] -= m_list1[t_i]
    loss = -mean_i(log_softmax(out)[i, t_i])

Decomposition: per row i with xt_i = x[i, t_i], m_i = m_list1[t_i],

    S0_i   = sum_j exp(x_ij)                       <- device (O(B*C))
    S_i    = S0_i - exp(xt_i) + exp(xt_i - m_i)    <- host (O(B))
    loss_i = log(S_i) - (xt_i - m_i)               <- host (O(B))

x ~ N(0,1), so exp(x) needs no max-subtraction in f32. The device does
the single O(B*C) pass — stream x once from HBM (the memory roofline)
— and returns per-row sums S0. The O(B) gathers, margin tables
("__init__" constants) and epilogue stay on the host.

Sharding: data-parallel, 4096 rows per core across 8 NeuronCores.

Device pipeline (final, all constants HW-measured from traces):
- PARTITION-MAJOR layout: partition p holds rows p*32..p*32+31 as one
  contiguous 128 KB HBM range, so each chunk DMA is one 4-16 KB
  contiguous descriptor per partition. Sustains 405-440 GB/s/core on
  HW vs ~350 GB/s for a row-major layout's 4 KB/row descriptors.
- Whole 16 MB f32 shard SBUF-resident (125 KB/partition) -> no buffer
  ring, no flow control: all 16 x dma_starts issue back-to-back (in
  the MAIN bb, before the block, overlapping the block-bb fetch).
- Per multi-slot chunk, ScalarE runs ONE fused exp+rowsum slot
  (activation with accum_out, ~1.6 us) then ONE plain multi-slot exp
  (0.91 us/slot: the (N+352)-cycle instruction overhead amortizes)
  writing bf16; VectorE reduce_sums the remaining slots (~1.07
  ns/elem, dtype-independent). The 1-fused+3-plain split keeps BOTH
  engines under the ~1.15 us/slot DMA delivery rate, so the kernel
  stays memory-bound and the post-stream tail is short.
- The ACT accumulator write-back lags retirement by a few
  instructions, so fused slots never sit near the end of the program
  (the final pieces all go through the DVE path, and the output DMA
  is additionally gated on the DVE sem).
- Chunk shape [4x6,3,2,1,1,.5,.5] row-slots: full-size 2 MB transfers
  from the first chunk (head pieces are issue-ramp-limited, not
  size-limited, and ScalarE has slack, so an early compute start buys
  nothing; pre-queued drain pieces run at 387-436 GB/s), small pieces
  at the drain end (the completion sem fires ~2.4 us after a chunk's
  last byte under load). Dummy activations as the FIRST instructions
  of the scalar block bb preload the ACT exp table during chunk 0's
  flight (ACT_TABLE_LOAD re-triggers per basic block, so a main-bb
  preload would leave a second 1.3 us load). The s0 output DMA is
  split: cols[0:30] flush as soon as col 29's reduce lands — the 9th
  DVE inc, when every fused-accum column is >=3 us past its
  write-back — so only the 12 B/partition cols[30:33] write sits on
  the critical path, followed by the mandatory completion wait
  (ending the block with a DMA in flight hard-crashes the device).
"""

import numpy as np

B, C = 32768, 1000
N_CORES = 8
R = B // N_CORES          # 4096 rows per core
P = 128                   # SBUF partitions
NT = R // P               # 32 row-slots per partition
CH = C // 2               # 500-column half piece

# full-slot chunks: slots 0..30 as [4*6, 3, 2, 1, 1] — full-size 2 MB
# transfers from the very first chunk (small head pieces measured only
# 174-397 GB/s and cost ~3 us of stream time; ScalarE has enough slack
# that an early start is unnecessary)
FULL_CHUNKS = [(0, 4), (4, 4), (8, 4), (12, 4), (16, 4), (20, 4),
               (24, 3), (27, 2), (29, 1), (30, 1)]
assert sum(s for _, s in FULL_CHUNKS) == NT - 1

# s0 column map: slots 0..30 -> cols 0..30; slot 31 halves -> cols
# 31,32 (written last, so the output DMA splits into an early
# cols[0:30] transfer whose issue and completion receipt overlap the
# slot-30/31 tail, and a tiny final cols[30:33] write); col 33 =
# preload scratch.
NCOLS = 34

ALPHA, POW_P, BETA = 0.5, 2.0, 0.3
E1, E2 = 60, 80
MAGIC = 0.165745444183859

_NC = None


def _build_nc():
    import concourse.bass as bass
    from concourse import mybir
    from contextlib import ExitStack

    f32 = mybir.dt.float32
    bf16 = mybir.dt.bfloat16
    Act = mybir.ActivationFunctionType

    nc = bass.Bass("TRN2", target_bir_lowering=False, debug=False,
                   num_devices=N_CORES)
    x = nc.dram_tensor("x", [R, C], f32, kind="ExternalInput")
    s0_d = nc.dram_tensor("s0", [P, NCOLS], f32, kind="ExternalOutput")

    # partition-major: partition p <- rows [p*NT, (p+1)*NT), so each
    # partition line is one contiguous 128 KB HBM range
    xv = x.ap().rearrange("(p t) c -> p t c", p=P)   # [128, 32, 1000]

    # (ap_fn, src, cols, fused) per piece: ap_fn slices both xbuf (exp
    # input) and ebuf (exp output / reduce input); cols = s0 column
    # range; fused = # leading slots reduced on ACT via accum_out (the
    # rest go through one plain multi-slot exp + a DVE reduce)
    pieces = []

    def half(t, h, col):
        ap = lambda buf: buf[:, t, h * CH:(h + 1) * CH]
        return (ap, xv[:, t, h * CH:(h + 1) * CH], (col, col + 1), 0)

    def run(t0, s):
        ap = lambda buf: buf[:, t0:t0 + s]
        return (ap, xv[:, t0:t0 + s], (t0, t0 + s), 1 if s > 1 else 0)

    for t0, s in FULL_CHUNKS:
        pieces.append(run(t0, s))
    pieces.append(half(NT - 1, 0, 31))
    pieces.append(half(NT - 1, 1, 32))
    NP = len(pieces)
    NDVE = sum(1 for (_, _, (c0, c1), f) in pieces if c1 - c0 > f)

    with ExitStack() as ctx:
        xbuf = ctx.enter_context(nc.sbuf_tensor([P, NT, C], f32))
        ebuf = ctx.enter_context(nc.sbuf_tensor([P, NT, C], bf16))
        s0 = ctx.enter_context(nc.sbuf_tensor([P, NCOLS], f32))

        sems = [ctx.enter_context(nc.semaphore(f"xc{i}"))
                for i in range(NP)]
        act_sem = ctx.enter_context(nc.semaphore("act_sem"))
        dve_sem = ctx.enter_context(nc.semaphore("dve_sem"))
        out_sem = ctx.enter_context(nc.semaphore("out_sem"))

        # issue every x-chunk DMA and the exp-table-preload dummy in the
        # MAIN bb, before the block: the engines then start the stream /
        # table load immediately after the entry barrier instead of
        # paying the ~1.6 us block-bb branch + instruction fetch first
        # (that fetch now overlaps the first chunks' flight)
        for (ap, src, _, _), sem in zip(pieces, sems):
            nc.sync.dma_start(ap(xbuf), src).then_inc(sem, 16)
        with nc.Block(no_gpsimd_drain=True) as block:

            @block.sync
            def _(sync):
                # DVE inc order: col 29's reduce is the 9th inc, so
                # after NDVE-3 every column 0..29 is final (fused cols
                # <= 27 are then >= 3 us past their accumulator
                # write-back); cols 30..32 are all DVE-written and
                # final after the last inc
                sync.wait_ge(dve_sem, NDVE - 3)
                sync.dma_start(s0_d.ap()[:, 0:30], s0[:, 0:30]) \
                    .then_inc(out_sem, 16)
                sync.wait_ge(dve_sem, NDVE)
                sync.dma_start(s0_d.ap()[:, 30:33], s0[:, 30:33]) \
                    .then_inc(out_sem, 16)
                sync.wait_ge(out_sem, 32)

            @block.scalar
            def _(scalar):
                # preload the ACT exp table during chunk 0's flight; as
                # the FIRST activation of this bb it absorbs the one
                # ACT_TABLE_LOAD (the load re-triggers per basic block,
                # so a main-bb preload leaves a second 1.3 us load
                # here). Col 33 of s0 is scratch; ebuf[:, 0, 0:2] is
                # overwritten by chunk 0's exps.
                scalar.activation(ebuf[:, 0, 0:1], s0[:, 33:34], Act.Exp)
                scalar.activation(ebuf[:, 0, 1:2], s0[:, 33:34], Act.Exp,
                                  accum_out=s0[:, 33:34])
                for (ap, _, (c0, c1), f), sem in zip(pieces, sems):
                    scalar.wait_ge(sem, 16)
                    for j in range(f):
                        # fused exp + row-sum for the leading slot(s);
                        # never within the last pieces, so the ACT
                        # accumulator write-back has long landed before
                        # the output DMA reads s0
                        t = c0 + j
                        scalar.activation(ebuf[:, t], xbuf[:, t], Act.Exp,
                                          accum_out=s0[:, t:t + 1])
                    if c1 - c0 > f:
                        if f:
                            scalar.activation(ebuf[:, c0 + f:c1],
                                              xbuf[:, c0 + f:c1], Act.Exp) \
                                .then_inc(act_sem)
                        else:
                            scalar.activation(ap(ebuf), ap(xbuf), Act.Exp) \
                                .then_inc(act_sem)

            @block.vector
            def _(vector):
                k = 0
                for (ap, _, (c0, c1), f) in pieces:
                    if c1 - c0 <= f:
                        continue
                    k += 1
                    vector.wait_ge(act_sem, k)
                    if f:
                        vector.reduce_sum(s0[:, c0 + f:c1],
                                          ebuf[:, c0 + f:c1],
                                          axis=mybir.AxisListType.X) \
                            .then_inc(dve_sem)
                    else:
                        vector.reduce_sum(s0[:, c0:c1], ap(ebuf),
                                          axis=mybir.AxisListType.X) \
                            .then_inc(dve_sem)
    return nc


def _get_nc():
    global _NC
    if _NC is None:
        _NC = _build_nc()
    return _NC


def _margins(cls_num_list, class_difficulty, epoch):
    cls = np.asarray(cls_num_list, dtype=np.float32)
    diff = np.asarray(class_difficulty, dtype=np.float32)
    max_m = np.float32(-np.log(cls.min() / cls.sum()) - np.float32(MAGIC))
    cls_p = (1.0 / np.sqrt(cls)).astype(np.float32)
    m_list = (max_m * cls_p / cls_p.max()).astype(np.float32)
    w = (ALPHA * diff ** POW_P + BETA).astype(np.float32)
    w = (w * (max_m / w.max())).astype(np.float32)
    ep = int(epoch)
    if ep < E1:
        m1 = m_list
    else:
        ee = 1.0 if ep >= E2 else (ep - E1) / (E2 - E1)
        m1 = (m_list + w * (ee / 2)).astype(np.float32)
    return m1


def _in_maps(x, targets, cls_num_list, class_difficulty, epoch):
    x = np.ascontiguousarray(np.asarray(x, dtype=np.float32))
    maps = [{"x": x[cid * R:(cid + 1) * R]} for cid in range(N_CORES)]
    return maps


def run_device(in_maps, trace=False, tmpdir=None):
    from concourse.bass_utils import run_bass_kernel_spmd
    kw = {}
    if trace:
        kw = dict(trace=True, tmpdir=tmpdir, trace_cores=list(range(N_CORES)))
    return run_bass_kernel_spmd(_get_nc(), in_maps,
                                core_ids=list(range(N_CORES)), **kw)


def _host_reference(x, tgt, m1):
    # numerically-stable fallback, never taken for the spec's randn inputs
    z = x.astype(np.float64).copy()
    rows = np.arange(B)
    z[rows, tgt] -= m1[tgt].astype(np.float64)
    mx = z.max(axis=1, keepdims=True)
    lse = np.log(np.exp(z - mx).sum(axis=1)) + mx[:, 0]
    return np.float32((lse - z[rows, tgt]).mean())


def kernel(x, targets, cls_num_list, class_difficulty, epoch):
    x = np.ascontiguousarray(np.asarray(x, dtype=np.float32))
    tgt = np.asarray(targets).astype(np.int64)
    m1 = _margins(cls_num_list, class_difficulty, epoch)
    if not np.isfinite(x).all() or np.abs(x).max() > 70.0:
        # exp without max-subtraction would overflow f32; spec fill is
        # randn so this never triggers in practice
        return _host_reference(x, tgt, m1)
    res = run_device(_in_maps(x, targets, cls_num_list,
                              class_difficulty, epoch))
    # partition-major: s0[p, j] = rowsum of local row p*32 + j; slot 31
    # arrives as half-row partial sums in cols (31, 32)
    parts = []
    for r in res.results:
        s = r["s0"].astype(np.float64)                         # [128, 34]
        full = np.empty((P, NT))
        full[:, 0:NT - 1] = s[:, 0:NT - 1]
        full[:, NT - 1] = s[:, 31] + s[:, 32]
        parts.append(full.reshape(-1))                         # local rows
    s0 = np.concatenate(parts)                                 # [B]
    xt = x[np.arange(B), tgt].astype(np.float64)
    m = m1[tgt].astype(np.float64)
    s = s0 - np.exp(xt) + np.exp(xt - m)
    loss = np.log(s) - (xt - m)
    return np.float32(loss.mean())
